# revision 2
# baseline (speedup 1.0000x reference)
"""Trainium2 Bass kernel for multi-head attention (b=4, n=2048, d=512, h=8, dk=dv=64).

Sharding: 8 cores = 4 batches x 2 query-halves. Each core computes K/V for its
full batch sequence (2048) and attention outputs for its 1024 query rows.
No collectives needed; host stacks the per-core [1024, 512] outputs.

The Scalar (ACT) engine is the roofline for this kernel: exp of the full
[8h x 2048j x 1024i] logit tensor is 131072 lane-columns at 1 elem/lane/cycle
@1.2GHz plus ~290cyc fixed cost per ACTIVATE = ~147us with [128,1024] tiles.
Everything else is scheduled to hide underneath it:

  ST row-tiling: dk=64 so the 128x128 PE array is split into two 64-row
    tiles (tile_position (0,0)/(64,0), auto-derived from base partitions).
    K^T is stored pair-packed: kt_p[i] = [128 = head2p dk | head2p+1 dk,
    2048 j]; qt likewise. The two heads' S^T matmuls run CONCURRENTLY on
    the two row-halves, writing the two psum banks of one [128, 1024] st
    tile ([S_A^T(jc) | S_B^T(jc)], 512 i cols each).  ST cost halves vs
    zero-padded 128-contraction (65536 vs 131072 cycles/core).
  One ACTIVATE per st tile ([128, 1024] f32 psum -> bf16 pt in SBUF); the
    Scalar queue carries NOTHING else (no copies, no DMA triggers).
  PV: per j-chunk, two [65, 512] matmuls (ones-column of V_aug gives the
    softmax denominator as psum row 64), accumulated over 16 j-chunks.
    PV lags ST by LAG slots so the exp semaphore is always satisfied
    when PV issues (a sem-blocked PV stalls the FIFO tensor queue).
  Projections (Q/K/V/y) are emitted via a work queue, ONE 4-matmul group
    per slot, so the tensor queue never runs a long projection burst that
    would starve the ACT engine (its st double-buffer covers ~1 slot).
  Input DMAs are consolidated (one per tensor) and strictly ordered on the
    sync queue: wq, xt[g0], wk, relb, xt[g1], wv, xt[g2], xt[g3], wo, bo.
    In-queue FIFO = true priority; first ST issues ~5us in (was ~13us with
    round-robin trigger serialization; ACT started at 24.7us).
  Normalization: reciprocal_approx_fast + gpsimd partition broadcast + DVE
    multiply -> outt (fp16, per head pair).  y = outt^T @ Wo + bo; i-blocks
    0-3 are projected as soon as (pair3, ih0) is normalized; the tail
    projects blocks 4-7 from the st psum pool (free after the last exp).

PSUM budget (8 banks): st pool 2 bufs x [128, 1024] f32 (2 banks each)
+ qk pool 1 buf x [128, 512] (1 bank) + pv 3 bufs x [65, 512] (3 banks).
NOTE: pool tiles allocated in loops must use a CONSTANT name= / tag= --
unique per-iteration names break buffer rotation.
"""
import numpy as np

B, N, MODEL = 4, 2048, 512
H, DK = 8, 64
SCALE = DK ** -0.5
NP = H // 2         # head pairs
NI = 1024           # query rows per core
NCH = MODEL // 128  # model-dim chunks
NJC = N // 128      # key/value chunks
LAG = 8             # PV lags ST by this many slots
TOT = NP * 2 * NJC  # 128 slots: (pair, ih, jc)

_COMPILED = None


def _build():
    import concourse.bass as bass
    from concourse import bacc
    import concourse.mybir as mybir
    import concourse.tile as tile
    from collections import deque

    F32 = mybir.dt.float32
    BF16 = mybir.dt.bfloat16
    FP16 = mybir.dt.float16
    EXP = mybir.ActivationFunctionType.Exp

    nc = bacc.Bacc("TRN2", target_bir_lowering=False, debug=False, num_devices=8)
    xt_in = nc.dram_tensor("xt", [MODEL, N], FP16, kind="ExternalInput")
    wq_in = nc.dram_tensor("wq", [MODEL, MODEL], FP16, kind="ExternalInput")
    wk_in = nc.dram_tensor("wk", [MODEL, MODEL], FP16, kind="ExternalInput")
    wv_in = nc.dram_tensor("wv", [MODEL, MODEL], FP16, kind="ExternalInput")
    relb_in = nc.dram_tensor("relb", [128, NP], F32, kind="ExternalInput")
    wo_in = nc.dram_tensor("wo", [MODEL, MODEL], FP16, kind="ExternalInput")
    bo_in = nc.dram_tensor("bo", [1, MODEL], F32, kind="ExternalInput")
    onesb_in = nc.dram_tensor("onesb", [128, NJC * H], BF16, kind="ExternalInput")
    y_out = nc.dram_tensor("y", [NI, MODEL], F32, kind="ExternalOutput")

    with tile.TileContext(nc) as tc:
        with (
            tc.tile_pool(name="w", bufs=1) as wp,
            tc.tile_pool(name="acts", bufs=1) as ap,
            tc.tile_pool(name="st", bufs=2, space="PSUM") as stp,
            tc.tile_pool(name="qk", bufs=1, space="PSUM") as qkp,
            tc.tile_pool(name="pv", bufs=3, space="PSUM") as pvp,
        ):
            # ---------- persistent tiles ----------
            wq = wp.tile([128, NCH, MODEL], FP16, tag="wq")
            wk = wp.tile([128, NCH, MODEL], FP16, tag="wk")
            wv = wp.tile([128, NCH, MODEL], FP16, tag="wv")
            wo = wp.tile([128, NP, MODEL], FP16, tag="wo")
            relb = wp.tile([128, NP], F32, tag="relb")
            bo = wp.tile([1, MODEL], F32, tag="bo")
            bo_b = wp.tile([128, MODEL], F32, tag="bo_b")
            onesb_t = wp.tile([128, NJC * H], BF16, tag="onesb")

            # x^T staged as [128, g*4+ch, 512]: g = 512-col group, ch = chunk
            xt = ap.tile([128, 16, 512], FP16, tag="xt")
            vv_t = [ap.tile([128, 4, H * 65], BF16, name=f"vv{i}",
                            tag=f"vv{i}") for i in range(4)]
            qt_t = [[ap.tile([128, 512], FP16, name=f"qt{i}{g}",
                             tag=f"qt{i}{g}") for g in range(2)]
                    for i in range(2)]
            # pair-packed K^T: partitions 0:64 = head 2p, 64:128 = head 2p+1
            kt_p = [ap.tile([128, NJC * 128], FP16, name=f"kt_p{i}",
                            tag=f"kt{i}") for i in range(2)]
            outt_p = [ap.tile([128, NI], FP16, name=f"outt{i}", tag=f"outt{i}")
                      for i in range(NP)]

            def vvt(jc):
                return vv_t[jc // 4][:, jc % 4]

            def r3(d):
                return d[:].rearrange("(c p) n -> p c n", p=128)

            def xtv(ch, start, size):
                g, off = start // 512, start % 512
                assert off + size <= 512
                return xt[:, g * 4 + ch, off:off + size]

            # ---------- input staging: one DMA per tensor, strict sync-queue
            # priority order (in-queue FIFO + full-BW per transfer) ----------
            xsrc = r3(xt_in)
            nc.sync.dma_start(out=wq[:], in_=r3(wq_in))
            nc.sync.dma_start(out=xt[:, 0:4, :], in_=xsrc[:, :, 0:512])
            nc.sync.dma_start(out=wk[:], in_=r3(wk_in))
            nc.sync.dma_start(out=relb[:], in_=relb_in[:])
            nc.sync.dma_start(out=xt[:, 4:8, :], in_=xsrc[:, :, 512:1024])
            nc.sync.dma_start(out=wv[:], in_=r3(wv_in))
            nc.sync.dma_start(out=xt[:, 8:12, :], in_=xsrc[:, :, 1024:1536])
            nc.sync.dma_start(out=xt[:, 12:16, :], in_=xsrc[:, :, 1536:2048])
            nc.sync.dma_start(out=wo[:], in_=r3(wo_in))
            nc.sync.dma_start(out=bo[:], in_=bo_in[:])
            nc.gpsimd.dma_start(out=onesb_t[:], in_=onesb_in[:])
            nc.gpsimd.partition_broadcast(bo_b[:], bo[:])
            # ones columns of V_aug: contiguous DMA to scratch, strided copy
            for vh in range(4):
                nc.gpsimd.tensor_copy(
                    vv_t[vh][:]
                    .rearrange("p j (h e) -> p (j h) e", e=65)[:, :, 64:65],
                    onesb_t[:, vh * 4 * H:(vh + 1) * 4 * H]
                    .rearrange("p (n o) -> p n o", o=1))

            with (
                tc.tile_pool(name="pt", bufs=LAG + 4) as ptp,
                tc.tile_pool(name="norm", bufs=2) as np_,
                tc.tile_pool(name="ysb", bufs=2) as yp_sb,
            ):
                # ---- projection group emitters (one 4-matmul group each) ----
                def emit_q(p, g):
                    q_ps = qkp.tile([128, 512], F32, name="qk", tag="qk")
                    cols = slice(p * 128, (p + 1) * 128)
                    for ch in range(NCH):
                        nc.tensor.matmul(
                            q_ps[:], wq[:, ch, cols], xtv(ch, g * 512, 512),
                            start=(ch == 0), stop=(ch == NCH - 1))
                    nc.vector.tensor_scalar_add(
                        qt_t[p % 2][g][:], q_ps[:], relb[:, p:p + 1])

                def emit_k(p, g):
                    k_ps = qkp.tile([128, 512], F32, name="qk", tag="qk")
                    cols = slice(p * 128, (p + 1) * 128)
                    for ch in range(NCH):
                        nc.tensor.matmul(
                            k_ps[:], wk[:, ch, cols], xtv(ch, g * 512, 512),
                            start=(ch == 0), stop=(ch == NCH - 1))
                    nc.vector.tensor_copy(
                        kt_p[p % 2][:, g * 512:(g + 1) * 512], k_ps[:])

                def emit_vjc(jc):
                    v_ps = qkp.tile([128, 512], F32, name="qk", tag="qk")
                    for ch in range(NCH):
                        nc.tensor.matmul(
                            v_ps[:], xtv(ch, jc * 128, 128), wv[:, ch],
                            start=(ch == 0), stop=(ch == NCH - 1))
                    nc.vector.tensor_copy(
                        vvt(jc).rearrange("p (h e) -> p h e", e=65)[:, :, 0:64],
                        v_ps[:].rearrange("p (h e) -> p h e", e=64))

                def emit_y(ib, pool):
                    y_ps = pool.tile([128, 512], F32, name="y_ps",
                                     tag="st" if pool is stp else "qk")
                    for hp2 in range(NP):
                        nc.tensor.matmul(
                            y_ps[:], outt_p[hp2][:, ib * 128:(ib + 1) * 128],
                            wo[:, hp2], start=(hp2 == 0), stop=(hp2 == NP - 1))
                    y_sb = yp_sb.tile([128, MODEL], F32, name="y_sb", tag="ysb")
                    nc.vector.tensor_tensor(out=y_sb[:], in0=y_ps[:],
                                            in1=bo_b[:], op=mybir.AluOpType.add)
                    (nc.sync if ib % 2 == 0 else nc.gpsimd).dma_start(
                        out=y_out[ib * 128:(ib + 1) * 128, :], in_=y_sb[:])

                proj_q = deque()

                def pump(n=1):
                    for _ in range(n):
                        if not proj_q:
                            return
                        proj_q.popleft()()

                # pair 0 prerequisites for slot 0 (emitted inline)
                emit_q(0, 0)
                emit_q(0, 1)
                emit_k(0, 0)
                # rest of pair 0's K + all V groups, interleaved so K group g
                # lands before ST needs it (slot 4g) and V group j before PV
                # needs it (slot LAG + j)
                proj_q.extend([
                    lambda: emit_k(0, 1), lambda: emit_vjc(0),
                    lambda: emit_vjc(1), lambda: emit_k(0, 2),
                    lambda: emit_vjc(2), lambda: emit_vjc(3),
                    lambda: emit_k(0, 3),
                ])
                proj_q.extend([(lambda j=j: emit_vjc(j)) for j in range(4, NJC)])

                pv_a = {}
                pv_b = {}
                pts = {}

                def finalize(u, pv_t, hr):
                    """normalize pv rows 0:64 by psum row 64 -> outt."""
                    p2, ih2 = u // 2, u % 2
                    hp2 = p2
                    isl = slice(ih2 * 512, (ih2 + 1) * 512)
                    den = np_.tile([1, 512], F32, name="den", tag="den")
                    nc.vector.tensor_copy(den[:], pv_t[64:65, :])
                    rrow = np_.tile([1, 512], F32, name="rrow", tag="rrow")
                    nc.vector.reciprocal_approx_fast(rrow[:], den[:])
                    rb = np_.tile([64, 512], F32, name="rb", tag="rb")
                    nc.gpsimd.partition_broadcast(rb[:], rrow[:])
                    nc.vector.tensor_tensor(
                        out=outt_p[hp2][hr:hr + 64, isl],
                        in0=pv_t[0:64, :], in1=rb[:],
                        op=mybir.AluOpType.mult)

                for g in range(TOT + LAG):
                    if g < TOT:
                        p, ih, jc = g // 32, (g // 16) % 2, g % 16
                        pi = p % 2
                        jsl = slice(jc * 128, (jc + 1) * 128)
                        st = stp.tile([128, 1024], F32, name="st", tag="st")
                        nc.tensor.matmul(st[:, 0:512], kt_p[pi][0:64, jsl],
                                         qt_t[pi][ih][0:64, :],
                                         start=True, stop=True)
                        nc.tensor.matmul(st[:, 512:1024], kt_p[pi][64:128, jsl],
                                         qt_t[pi][ih][64:128, :],
                                         start=True, stop=True)
                        pt = ptp.tile([128, 1024], BF16, name="pt", tag="pt")
                        pts[g] = pt
                        nc.scalar.activation(pt[:], st[:], EXP, scale=1.0)
                        # queue next pair's projections once this pair's ST
                        # stream owns the qt/kt ping-pong tiles
                        if g % 32 == 2 and p + 1 < NP:
                            pn = p + 1
                            proj_q.extend([
                                (lambda pp=pn: emit_q(pp, 0)),
                                (lambda pp=pn: emit_q(pp, 1)),
                                (lambda pp=pn: emit_k(pp, 0)),
                                (lambda pp=pn: emit_k(pp, 1)),
                                (lambda pp=pn: emit_k(pp, 2)),
                                (lambda pp=pn: emit_k(pp, 3)),
                            ])
                        pump(1)
                    if g >= LAG:
                        gp = g - LAG
                        u, jc2 = gp // 16, gp % 16
                        p2 = u // 2
                        hA, hB = 2 * p2, 2 * p2 + 1
                        if jc2 == 0:
                            pv_a[u] = pvp.tile([65, 512], F32, name="pva",
                                               tag="pv")
                            pv_b[u] = pvp.tile([65, 512], F32, name="pvb",
                                               tag="pv")
                        ptb = pts.pop(gp)
                        nc.tensor.matmul(pv_a[u][:],
                                         vvt(jc2)[:, hA * 65:(hA + 1) * 65],
                                         ptb[:, 0:512],
                                         start=(jc2 == 0), stop=(jc2 == 15))
                        nc.tensor.matmul(pv_b[u][:],
                                         vvt(jc2)[:, hB * 65:(hB + 1) * 65],
                                         ptb[:, 512:1024],
                                         start=(jc2 == 0), stop=(jc2 == 15))
                        if jc2 == 15:
                            finalize(u, pv_a.pop(u), 0)
                            finalize(u, pv_b.pop(u), 64)
                            if u == 6:
                                # (pair3, ih0) done: i-blocks 0-3 ready
                                proj_q.extend([
                                    (lambda b=b: emit_y(b, qkp))
                                    for b in range(4)])
                            if u == 7:
                                for b in range(4, 8):
                                    emit_y(b, stp)
                    if g >= TOT:
                        pump(1)

    nc.compile()
    return nc


def _get_compiled():
    global _COMPILED
    if _COMPILED is None:
        _COMPILED = _build()
    return _COMPILED


def kernel(x, Wq, Wk, Wv, Wo, bo, rel_content_bias, _trace=False):
    from concourse.bass_utils import run_bass_kernel_spmd
    import ml_dtypes

    nc = _get_compiled()

    x = np.asarray(x, dtype=np.float32)
    Wq = np.asarray(Wq, dtype=np.float32)
    Wk = np.asarray(Wk, dtype=np.float32)
    Wv = np.asarray(Wv, dtype=np.float32)
    Wo = np.asarray(Wo, dtype=np.float32)
    bo = np.asarray(bo, dtype=np.float32)
    bias = np.asarray(rel_content_bias, dtype=np.float32).reshape(H, DK)

    Wq_s = (Wq * SCALE).astype(np.float32)
    # relb column p = [bias of head 2p (64) | bias of head 2p+1 (64)]
    relb = bias.reshape(NP, 2 * DK).T.astype(np.float32)  # [128, NP]
    onesb = np.ones((128, NJC * H), ml_dtypes.bfloat16)
    shared = {"wq": Wq_s.astype(np.float16), "wk": Wk.astype(np.float16),
              "wv": Wv.astype(np.float16), "relb": relb,
              "wo": Wo.astype(np.float16), "bo": bo[None, :], "onesb": onesb}

    in_maps = []
    for c in range(8):
        b, half = c // 2, c % 2
        xt = np.ascontiguousarray(x[b].T).astype(np.float16)   # [512, 2048]
        if half:
            xt = np.ascontiguousarray(np.roll(xt, -NI, axis=1))
        in_maps.append({"xt": xt, **shared})

    res = run_bass_kernel_spmd(nc, in_maps, core_ids=list(range(8)),
                               trace=_trace)
    out = np.empty((B, N, MODEL), np.float32)
    for c in range(8):
        b, half = c // 2, c % 2
        out[b, half * NI:(half + 1) * NI, :] = res.results[c]["y"]
    if _trace:
        return out, res
    return out
